# revision 13
# baseline (speedup 1.0000x reference)
"""Trainium2 Bass kernel for nn_MemoryBlock (scatter_memory).

Mathematical identity: softmax over the memory-unit axis U produces rows
that sum to exactly 1, and the output is

    out[b] = relu( mean_u( sum_n attn[b,n,u] * V[b,n,:] ) @ Wo + bo )
           = relu( ((1/U) * sum_n V[b,n,:]) @ Wo + bo )
           = relu( ((sum_n X[b,n,:]) @ Wv + N*bv) / U @ Wo + bo )

so the K/scores/softmax path cancels algebraically.  The kernel computes a
memory-bound column-sum of X per batch (the roofline: reading all of X),
then two tiny matmuls on-device.

Sharding: data-parallel over batch B=16 across 8 cores (2 batches/core);
the small weights are replicated.  Per core: DMA 16.8 MB of X, reduce
over N with ones-vector matmuls on the TensorEngine accumulating in PSUM,
transpose the per-batch sums into columns on the PE, then
out0T = Wv'.T @ ST + bv', outT = relu(Wo.T @ out0T + bo).
"""

import numpy as np

B, N, FEAT, MEM, U = 16, 8192, 256, 128, 512
NCORES = 8
BPC = B // NCORES  # batches per core

CH = 16            # rows-per-partition per DMA chunk -> [128, CH*FEAT] = 2 MB
RPP = N // 128     # rows per partition per batch
NCH = RPP // CH    # DMA chunks per batch
MMW = 512          # matmul moving-operand free width (PSUM bank limit, fp32)

_built = None


def _ensure_axon_hooks():
    """Provide antenv.axon_hooks if the image lacks it (trace plumbing)."""
    try:
        import antenv.axon_hooks  # noqa: F401
        return
    except ImportError:
        pass
    import sys
    import types

    m = types.ModuleType("antenv.axon_hooks")
    holder = [None]
    m.set_axon_ntff_profile_hook = lambda h: holder.__setitem__(0, h)
    m.get_axon_ntff_profile_hook = lambda: holder[0]
    sys.modules["antenv.axon_hooks"] = m
    try:
        import antenv

        antenv.axon_hooks = m
    except ImportError:
        pass


def _build():
    import concourse.bacc as bacc
    import concourse.mybir as mybir
    from concourse.tile import TileContext

    f32 = mybir.dt.float32
    nc = bacc.Bacc(None)

    X_d = nc.dram_tensor("Xs", [BPC, N, FEAT], f32, kind="ExternalInput")
    Wv_d = nc.dram_tensor("Wvs", [2, 128, MEM], f32, kind="ExternalInput")
    Wo_d = nc.dram_tensor("Wos", [MEM, MEM], f32, kind="ExternalInput")
    bv_d = nc.dram_tensor("bvc", [MEM, 1], f32, kind="ExternalInput")
    bo_d = nc.dram_tensor("boc", [MEM, 1], f32, kind="ExternalInput")
    out_d = nc.dram_tensor("outT", [MEM, BPC], f32, kind="ExternalOutput")

    with TileContext(nc) as tc:
        with (
            tc.tile_pool(name="const", bufs=1) as cpool,
            tc.tile_pool(name="xp", bufs=BPC * NCH) as xpool,
            tc.tile_pool(name="sp", bufs=2) as spool,
            tc.tile_pool(name="psacc", bufs=2, space="PSUM") as ps_acc,
            tc.tile_pool(name="pssm", bufs=4, space="PSUM") as ps_sm,
        ):
            ones = cpool.tile([128, 1], f32)
            nc.vector.memset(ones[:, :], 1.0)
            wv_sb = cpool.tile([128, 2 * MEM], f32)
            nc.sync.dma_start(out=wv_sb[:, 0:MEM], in_=Wv_d[0])
            nc.sync.dma_start(out=wv_sb[:, MEM : 2 * MEM], in_=Wv_d[1])
            wo_sb = cpool.tile([128, MEM], f32)
            nc.sync.dma_start(out=wo_sb[:, :], in_=Wo_d[:, :])
            bv_sb = cpool.tile([128, 1], f32)
            nc.sync.dma_start(out=bv_sb[:, :], in_=bv_d[:, :])
            bo_sb = cpool.tile([128, 1], f32)
            nc.sync.dma_start(out=bo_sb[:, :], in_=bo_d[:, :])

            # ST columns per feature-half: st[h][:, b] = colsum(X[b])[h*128:(h+1)*128]
            st = [
                cpool.tile([128, BPC], f32, tag=f"st{h}", name=f"st{h}")
                for h in range(2)
            ]

            # Warmup reads of the bias columns on the engines that consume
            # them (DVE: tensor_scalar_add, ACT: relu bias) so those DMA
            # waits are absorbed ahead of ops that also wait on the PE.
            dve_scr = spool.tile([1, 1], f32, tag="scr", bufs=1)
            nc.vector.tensor_copy(out=dve_scr[:, :], in_=bv_sb[0:1, 0:1])
            act_scr = spool.tile([1, 1], f32, tag="scr2", bufs=1)
            nc.scalar.activation(
                out=act_scr[:, :],
                in_=bo_sb[0:1, 0:1],
                func=mybir.ActivationFunctionType.Copy,
                scale=1.0,
            )

            # Warmup 1x1 matmuls (one accumulation group, result unused):
            # each absorbs one cross-engine wait on the PE (DVE memset, the
            # three weight DMAs) so every real matmul's LDWEIGHTS carries at
            # most one sync wait (the ISA struct has a single wait slot).
            warm = ps_sm.tile([1, 1], f32, tag="warm", bufs=1)
            warm_srcs = [
                ones[0:1, 0:1],
                wv_sb[0:1, 0:1],
                wv_sb[0:1, MEM : MEM + 1],
                wo_sb[0:1, 0:1],
            ]
            for wi, wap in enumerate(warm_srcs):
                nc.tensor.matmul(
                    warm[:, :],
                    lhsT=wap,
                    rhs=ones[0:1, 0:1],
                    start=(wi == 0),
                    stop=(wi == len(warm_srcs) - 1),
                )

            for b in range(BPC):
                ps = ps_acc.tile([1, MMW], f32, tag="ps")
                Xb = X_d[b].rearrange("(p r) f -> p (r f)", p=128)
                nmm = NCH * (CH * FEAT // MMW)
                k = 0
                for c in range(NCH):
                    xt = xpool.tile([128, CH * FEAT], f32, tag="xt")
                    nc.sync.dma_start(
                        out=xt[:, :], in_=Xb[:, c * CH * FEAT : (c + 1) * CH * FEAT]
                    )
                    for m in range(CH * FEAT // MMW):
                        nc.tensor.matmul(
                            ps[:, :],
                            lhsT=ones[:, :],
                            rhs=xt[:, m * MMW : (m + 1) * MMW],
                            start=(k == 0),
                            stop=(k == nmm - 1),
                        )
                        k += 1
                # psum row = [even-row partial | odd-row partial]; copy to SBUF
                # (only one PSUM operand allowed per DVE op), then fold halves
                s_row = spool.tile([1, 2 * FEAT], f32, tag="srow")
                nc.scalar.activation(
                    out=s_row[:, :],
                    in_=ps[0:1, :],
                    func=mybir.ActivationFunctionType.Copy,
                    scale=1.0,
                )
                s_sb = spool.tile([1, FEAT], f32, tag="s")
                nc.vector.tensor_add(
                    out=s_sb[:, :],
                    in0=s_row[0:1, 0:FEAT],
                    in1=s_row[0:1, FEAT : 2 * FEAT],
                )
                # transpose the [1,128] halves into PSUM columns on the PE
                for h in range(2):
                    pt = ps_sm.tile([128, 1], f32, tag="sm")
                    nc.tensor.transpose(
                        pt[:, :], s_sb[0:1, h * 128 : (h + 1) * 128], ones[0:1, 0:1]
                    )
                    nc.vector.tensor_copy(out=st[h][:, b : b + 1], in_=pt[:, :])

            # out0T = Wv'.T @ ST + bv'   (Wv' = Wv/U, bv' = bv*N/U, host-folded)
            psv = ps_sm.tile([128, BPC], f32, tag="sm")
            nc.tensor.matmul(
                psv[:, :], lhsT=wv_sb[:, 0:MEM], rhs=st[0][:, :], start=True, stop=False
            )
            nc.tensor.matmul(
                psv[:, :],
                lhsT=wv_sb[:, MEM : 2 * MEM],
                rhs=st[1][:, :],
                start=False,
                stop=True,
            )
            out0 = spool.tile([128, BPC], f32, tag="o0")
            nc.vector.tensor_scalar_add(out=out0[:, :], in0=psv[:, :], scalar1=bv_sb[:, 0:1])

            # outT = relu(Wo.T @ out0T + bo)
            pso = ps_sm.tile([128, BPC], f32, tag="sm")
            nc.tensor.matmul(pso[:, :], lhsT=wo_sb[:, :], rhs=out0[:, :], start=True, stop=True)
            res = spool.tile([128, BPC], f32, tag="res")
            import concourse.mybir as mybir2

            nc.scalar.activation(
                out=res[:, :],
                in_=pso[:, :],
                func=mybir2.ActivationFunctionType.Relu,
                bias=bo_sb[:, 0:1],
                scale=1.0,
            )
            nc.gpsimd.dma_start(out=out_d[:, :], in_=res[:, :])

    if not nc.is_finalized():
        nc.finalize()
    return nc


def kernel(X, mem, Wk, bk, Wv, bv, Wo, bo):
    global _built
    _ensure_axon_hooks()
    from concourse.bass_utils import run_bass_kernel_spmd

    if _built is None:
        _built = _build()
    nc = _built

    X = np.asarray(X, dtype=np.float32)
    Wvs = np.ascontiguousarray(
        (np.asarray(Wv, dtype=np.float32) / float(U)).reshape(2, 128, MEM)
    )
    Wos = np.ascontiguousarray(np.asarray(Wo, dtype=np.float32))
    bvc = np.ascontiguousarray(
        (np.asarray(bv, dtype=np.float32) * (N / float(U))).reshape(MEM, 1)
    )
    boc = np.ascontiguousarray(np.asarray(bo, dtype=np.float32).reshape(MEM, 1))

    in_maps = [
        {
            "Xs": np.ascontiguousarray(X[i * BPC : (i + 1) * BPC]),
            "Wvs": Wvs,
            "Wos": Wos,
            "bvc": bvc,
            "boc": boc,
        }
        for i in range(NCORES)
    ]
    r = run_bass_kernel_spmd(nc, in_maps, list(range(NCORES)))
    kernel._last_results = r

    out = np.empty((B, MEM), dtype=np.float32)
    for i in range(NCORES):
        out[i * BPC : (i + 1) * BPC] = r.results[i]["outT"].T
    return out


# revision 16
# speedup vs baseline: 1.1194x; 1.1194x over previous
"""Trainium2 Bass kernel for nn_MemoryBlock (scatter_memory).

Mathematical identity: softmax over the memory-unit axis U produces rows
that sum to exactly 1, and the output is

    out[b] = relu( mean_u( sum_n attn[b,n,u] * V[b,n,:] ) @ Wo + bo )
           = relu( ((1/U) * sum_n V[b,n,:]) @ Wo + bo )
           = relu( ((sum_n X[b,n,:]) @ Wv + N*bv) / U @ Wo + bo )

so the K/scores/softmax path cancels algebraically.  The kernel computes a
memory-bound column-sum of X per batch (the roofline: reading all of X),
then two tiny matmuls on-device.

Sharding: data-parallel over batch B=16 across 8 cores (2 batches/core);
the small weights are replicated.  Per core: DMA 16.8 MB of X, reduce
over N with ones-vector matmuls on the TensorEngine accumulating in PSUM,
transpose the per-batch sums into columns on the PE, then
out0T = Wv'.T @ ST + bv', outT = relu(Wo.T @ out0T + bo).
"""

import numpy as np

B, N, FEAT, MEM, U = 16, 8192, 256, 128, 512
NCORES = 8
BPC = B // NCORES  # batches per core

CH = 16            # rows-per-partition per DMA chunk -> [128, CH*FEAT] = 2 MB
RPP = N // 128     # rows per partition per batch
NCH = RPP // CH    # DMA chunks per batch
MMW = 512          # matmul moving-operand free width (PSUM bank limit, fp32)

_built = None


def _ensure_axon_hooks():
    """Provide antenv.axon_hooks if the image lacks it (trace plumbing)."""
    try:
        import antenv.axon_hooks  # noqa: F401
        return
    except ImportError:
        pass
    import sys
    import types

    m = types.ModuleType("antenv.axon_hooks")
    holder = [None]
    m.set_axon_ntff_profile_hook = lambda h: holder.__setitem__(0, h)
    m.get_axon_ntff_profile_hook = lambda: holder[0]
    sys.modules["antenv.axon_hooks"] = m
    try:
        import antenv

        antenv.axon_hooks = m
    except ImportError:
        pass


def _build():
    import concourse.bacc as bacc
    import concourse.mybir as mybir
    from concourse.tile import TileContext

    f32 = mybir.dt.float32
    f32r = mybir.dt.float32r  # fp32 bits, 1 cycle/row PE stream (vs 4 for fp32)
    nc = bacc.Bacc(None)

    X_d = nc.dram_tensor("Xs", [BPC, N, FEAT], f32r, kind="ExternalInput")
    Wv_d = nc.dram_tensor("Wvs", [2, 128, MEM], f32, kind="ExternalInput")
    Wo_d = nc.dram_tensor("Wos", [MEM, MEM], f32, kind="ExternalInput")
    bv_d = nc.dram_tensor("bvc", [MEM, 1], f32, kind="ExternalInput")
    bo_d = nc.dram_tensor("boc", [MEM, 1], f32, kind="ExternalInput")
    ones_d = nc.dram_tensor("onesc", [128, 1], f32r, kind="ExternalInput")
    out_d = nc.dram_tensor("outT", [MEM, BPC], f32, kind="ExternalOutput")

    with TileContext(nc) as tc:
        with (
            tc.tile_pool(name="const", bufs=1) as cpool,
            tc.tile_pool(name="xp", bufs=BPC * NCH) as xpool,
            tc.tile_pool(name="sp", bufs=2) as spool,
            tc.tile_pool(name="psacc", bufs=2, space="PSUM") as ps_acc,
            tc.tile_pool(name="pssm", bufs=4, space="PSUM") as ps_sm,
        ):
            ones = cpool.tile([128, 1], f32r)
            nc.sync.dma_start(out=ones[:, :], in_=ones_d[:, :])
            one_f = cpool.tile([1, 1], f32)
            nc.vector.memset(one_f[:, :], 1.0)
            wv_sb = cpool.tile([128, 2 * MEM], f32)
            nc.sync.dma_start(out=wv_sb[:, 0:MEM], in_=Wv_d[0])
            nc.sync.dma_start(out=wv_sb[:, MEM : 2 * MEM], in_=Wv_d[1])
            wo_sb = cpool.tile([128, MEM], f32)
            nc.sync.dma_start(out=wo_sb[:, :], in_=Wo_d[:, :])
            bv_sb = cpool.tile([128, 1], f32)
            nc.sync.dma_start(out=bv_sb[:, :], in_=bv_d[:, :])
            bo_sb = cpool.tile([128, 1], f32)
            nc.sync.dma_start(out=bo_sb[:, :], in_=bo_d[:, :])

            # ST columns per feature-half: st[h][:, b] = colsum(X[b])[h*128:(h+1)*128]
            st = [
                cpool.tile([128, BPC], f32, tag=f"st{h}", name=f"st{h}")
                for h in range(2)
            ]

            # Warmup reads of the bias columns on the engines that consume
            # them (DVE: tensor_scalar_add, ACT: relu bias) so those DMA
            # waits are absorbed ahead of ops that also wait on the PE.
            dve_scr = spool.tile([1, 1], f32, tag="scr", bufs=1)
            nc.vector.tensor_copy(out=dve_scr[:, :], in_=bv_sb[0:1, 0:1])
            act_scr = spool.tile([1, 1], f32, tag="scr2", bufs=1)
            nc.scalar.activation(
                out=act_scr[:, :],
                in_=bo_sb[0:1, 0:1],
                func=mybir.ActivationFunctionType.Copy,
                scale=1.0,
            )

            # Warmup 1x1 matmuls (one accumulation group, result unused):
            # each absorbs one cross-engine wait on the PE (DVE memset, the
            # three weight DMAs) so every real matmul's LDWEIGHTS carries at
            # most one sync wait (the ISA struct has a single wait slot).
            warm = ps_sm.tile([1, 1], f32, tag="warm", bufs=1)
            warm_srcs = [
                (wv_sb[0:1, 0:1], one_f[0:1, 0:1]),
                (wv_sb[0:1, MEM : MEM + 1], one_f[0:1, 0:1]),
                (wo_sb[0:1, 0:1], one_f[0:1, 0:1]),
            ]
            for wi, (wap, wrhs) in enumerate(warm_srcs):
                nc.tensor.matmul(
                    warm[:, :],
                    lhsT=wap,
                    rhs=wrhs,
                    start=(wi == 0),
                    stop=(wi == len(warm_srcs) - 1),
                )

            for b in range(BPC):
                ps = ps_acc.tile([1, MMW], f32, tag="ps")
                Xb = X_d[b].rearrange("(p r) f -> p (r f)", p=128)
                nmm = NCH * (CH * FEAT // MMW)
                k = 0
                for c in range(NCH):
                    xt = xpool.tile([128, CH * FEAT], f32r, tag="xt")
                    nc.sync.dma_start(
                        out=xt[:, :], in_=Xb[:, c * CH * FEAT : (c + 1) * CH * FEAT]
                    )
                    for m in range(CH * FEAT // MMW):
                        nc.tensor.matmul(
                            ps[:, :],
                            lhsT=ones[:, :],
                            rhs=xt[:, m * MMW : (m + 1) * MMW],
                            start=(k == 0),
                            stop=(k == nmm - 1),
                        )
                        k += 1
                # psum row = [even-row partial | odd-row partial]; copy to SBUF
                # (only one PSUM operand allowed per DVE op), then fold halves
                s_row = spool.tile([1, 2 * FEAT], f32, tag="srow")
                nc.scalar.activation(
                    out=s_row[:, :],
                    in_=ps[0:1, :],
                    func=mybir.ActivationFunctionType.Copy,
                    scale=1.0,
                )
                s_sb = spool.tile([1, FEAT], f32, tag="s")
                nc.vector.tensor_add(
                    out=s_sb[:, :],
                    in0=s_row[0:1, 0:FEAT],
                    in1=s_row[0:1, FEAT : 2 * FEAT],
                )
                # transpose the [1,128] halves into PSUM columns on the PE
                for h in range(2):
                    pt = ps_sm.tile([128, 1], f32, tag="sm")
                    nc.tensor.transpose(
                        pt[:, :], s_sb[0:1, h * 128 : (h + 1) * 128], one_f[0:1, 0:1]
                    )
                    nc.vector.tensor_copy(out=st[h][:, b : b + 1], in_=pt[:, :])

            # out0T = Wv'.T @ ST + bv'   (Wv' = Wv/U, bv' = bv*N/U, host-folded)
            psv = ps_sm.tile([128, BPC], f32, tag="sm")
            nc.tensor.matmul(
                psv[:, :], lhsT=wv_sb[:, 0:MEM], rhs=st[0][:, :], start=True, stop=False
            )
            nc.tensor.matmul(
                psv[:, :],
                lhsT=wv_sb[:, MEM : 2 * MEM],
                rhs=st[1][:, :],
                start=False,
                stop=True,
            )
            out0 = spool.tile([128, BPC], f32, tag="o0")
            nc.vector.tensor_scalar_add(out=out0[:, :], in0=psv[:, :], scalar1=bv_sb[:, 0:1])

            # outT = relu(Wo.T @ out0T + bo)
            pso = ps_sm.tile([128, BPC], f32, tag="sm")
            nc.tensor.matmul(pso[:, :], lhsT=wo_sb[:, :], rhs=out0[:, :], start=True, stop=True)
            res = spool.tile([128, BPC], f32, tag="res")
            import concourse.mybir as mybir2

            nc.scalar.activation(
                out=res[:, :],
                in_=pso[:, :],
                func=mybir2.ActivationFunctionType.Relu,
                bias=bo_sb[:, 0:1],
                scale=1.0,
            )
            nc.gpsimd.dma_start(out=out_d[:, :], in_=res[:, :])

    if not nc.is_finalized():
        nc.finalize()
    return nc


def kernel(X, mem, Wk, bk, Wv, bv, Wo, bo):
    global _built
    _ensure_axon_hooks()
    from concourse.bass_utils import run_bass_kernel_spmd

    if _built is None:
        _built = _build()
    nc = _built

    X = np.asarray(X, dtype=np.float32)
    Wvs = np.ascontiguousarray(
        (np.asarray(Wv, dtype=np.float32) / float(U)).reshape(2, 128, MEM)
    )
    Wos = np.ascontiguousarray(np.asarray(Wo, dtype=np.float32))
    bvc = np.ascontiguousarray(
        (np.asarray(bv, dtype=np.float32) * (N / float(U))).reshape(MEM, 1)
    )
    boc = np.ascontiguousarray(np.asarray(bo, dtype=np.float32).reshape(MEM, 1))
    onesc = np.ones((128, 1), dtype=np.float32)

    in_maps = [
        {
            "Xs": np.ascontiguousarray(X[i * BPC : (i + 1) * BPC]),
            "Wvs": Wvs,
            "Wos": Wos,
            "bvc": bvc,
            "boc": boc,
            "onesc": onesc,
        }
        for i in range(NCORES)
    ]
    r = run_bass_kernel_spmd(nc, in_maps, list(range(NCORES)))
    kernel._last_results = r

    out = np.empty((B, MEM), dtype=np.float32)
    for i in range(NCORES):
        out[i * BPC : (i + 1) * BPC] = r.results[i]["outT"].T
    return out


# revision 17
# speedup vs baseline: 1.3294x; 1.1877x over previous
"""Trainium2 Bass kernel for nn_MemoryBlock (scatter_memory).

Mathematical identity: softmax over the memory-unit axis U produces rows
that sum to exactly 1, and the output is

    out[b] = relu( mean_u( sum_n attn[b,n,u] * V[b,n,:] ) @ Wo + bo )
           = relu( ((1/U) * sum_n V[b,n,:]) @ Wo + bo )
           = relu( ((sum_n X[b,n,:]) @ Wv + N*bv) / U @ Wo + bo )

so the K/scores/softmax path cancels algebraically.  The kernel computes a
memory-bound column-sum of X per batch (the roofline: reading all of X),
then two tiny matmuls on-device.

Sharding: data-parallel over batch B=16 across 8 cores (2 batches/core);
the small weights are replicated.  Per core: DMA 16.8 MB of X, reduce
over N with ones-vector matmuls on the TensorEngine accumulating in PSUM,
transpose the per-batch sums into columns on the PE, then
out0T = Wv'.T @ ST + bv', outT = relu(Wo.T @ out0T + bo).
"""

import numpy as np

B, N, FEAT, MEM, U = 16, 8192, 256, 128, 512
NCORES = 8
BPC = B // NCORES  # batches per core

CH = 16            # rows-per-partition per DMA chunk -> [128, CH*FEAT] = 2 MB
RPP = N // 128     # rows per partition per batch
NCH = RPP // CH    # DMA chunks per batch
MMW = 512          # matmul moving-operand free width (PSUM bank limit, fp32)

_built = None


def _ensure_axon_hooks():
    """Provide antenv.axon_hooks if the image lacks it (trace plumbing)."""
    try:
        import antenv.axon_hooks  # noqa: F401
        return
    except ImportError:
        pass
    import sys
    import types

    m = types.ModuleType("antenv.axon_hooks")
    holder = [None]
    m.set_axon_ntff_profile_hook = lambda h: holder.__setitem__(0, h)
    m.get_axon_ntff_profile_hook = lambda: holder[0]
    sys.modules["antenv.axon_hooks"] = m
    try:
        import antenv

        antenv.axon_hooks = m
    except ImportError:
        pass


def _build():
    import concourse.bacc as bacc
    import concourse.mybir as mybir
    from concourse.tile import TileContext

    f32 = mybir.dt.float32
    f32r = mybir.dt.float32r  # fp32 bits, 1 cycle/row PE stream (vs 4 for fp32)
    nc = bacc.Bacc(None)

    X_d = nc.dram_tensor("Xs", [BPC, N, FEAT], f32r, kind="ExternalInput")
    Wv_d = nc.dram_tensor("Wvs", [2, 128, MEM], f32, kind="ExternalInput")
    Wo_d = nc.dram_tensor("Wos", [MEM, MEM], f32, kind="ExternalInput")
    bv_d = nc.dram_tensor("bvc", [MEM, 1], f32, kind="ExternalInput")
    bo_d = nc.dram_tensor("boc", [MEM, 1], f32, kind="ExternalInput")
    ones_d = nc.dram_tensor("onesc", [128, 1], f32r, kind="ExternalInput")
    out_d = nc.dram_tensor("outT", [MEM, BPC], f32, kind="ExternalOutput")

    with TileContext(nc) as tc:
        with (
            tc.tile_pool(name="const", bufs=1) as cpool,
            tc.tile_pool(name="xp", bufs=BPC * NCH) as xpool,
            tc.tile_pool(name="sp", bufs=2) as spool,
            tc.tile_pool(name="psacc", bufs=2, space="PSUM") as ps_acc,
            tc.tile_pool(name="pssm", bufs=4, space="PSUM") as ps_sm,
        ):
            # X chunk DMAs are emitted first (SP HWDGE ring) so the big
            # stream leads the ring; consts go on the ACT HWDGE ring.
            xts = []
            for b in range(BPC):
                Xb = X_d[b].rearrange("(p r) f -> p (r f)", p=128)
                for c in range(NCH):
                    xt = xpool.tile([128, CH * FEAT], f32r, tag="xt",
                                    name=f"xt{b}_{c}")
                    nc.sync.dma_start(
                        out=xt[:, :], in_=Xb[:, c * CH * FEAT : (c + 1) * CH * FEAT]
                    )
                    xts.append(xt)

            ones = cpool.tile([128, 1], f32r)
            nc.scalar.dma_start(out=ones[:, :], in_=ones_d[:, :])
            one_f = cpool.tile([1, 1], f32)
            nc.vector.memset(one_f[:, :], 1.0)
            wv_sb = cpool.tile([128, 2 * MEM], f32)
            nc.scalar.dma_start(out=wv_sb[:, 0:MEM], in_=Wv_d[0])
            nc.scalar.dma_start(out=wv_sb[:, MEM : 2 * MEM], in_=Wv_d[1])
            wo_sb = cpool.tile([128, MEM], f32)
            nc.scalar.dma_start(out=wo_sb[:, :], in_=Wo_d[:, :])
            bv_sb = cpool.tile([128, 1], f32)
            nc.scalar.dma_start(out=bv_sb[:, :], in_=bv_d[:, :])
            bo_sb = cpool.tile([128, 1], f32)
            nc.scalar.dma_start(out=bo_sb[:, :], in_=bo_d[:, :])

            # stq columns, h-major: (h0b0, h0b1, h1b0, h1b1) so phase-2's
            # rhs per feature-half is the contiguous pair stq[:, 2h:2h+2]
            stq = cpool.tile([128, 2 * BPC], f32)

            # Warmup reads of the bias columns on the engines that consume
            # them (DVE: tensor_scalar_add, ACT: relu bias) so those DMA
            # waits are absorbed ahead of ops that also wait on the PE.
            dve_scr = spool.tile([1, 1], f32, tag="scr", bufs=1)
            nc.vector.tensor_copy(out=dve_scr[:, :], in_=bv_sb[0:1, 0:1])
            act_scr = spool.tile([1, 1], f32, tag="scr2", bufs=1)
            nc.scalar.activation(
                out=act_scr[:, :],
                in_=bo_sb[0:1, 0:1],
                func=mybir.ActivationFunctionType.Copy,
                scale=1.0,
            )

            # Warmup 1x1 matmuls (one accumulation group, result unused):
            # each absorbs one cross-engine wait on the PE (DVE memset, the
            # three weight DMAs) so every real matmul's LDWEIGHTS carries at
            # most one sync wait (the ISA struct has a single wait slot).
            warm = ps_sm.tile([1, 1], f32, tag="warm", bufs=1)
            warm_srcs = [
                (wv_sb[0:1, 0:1], one_f[0:1, 0:1]),
                (wv_sb[0:1, MEM : MEM + 1], one_f[0:1, 0:1]),
                (wo_sb[0:1, 0:1], one_f[0:1, 0:1]),
            ]
            for wi, (wap, wrhs) in enumerate(warm_srcs):
                nc.tensor.matmul(
                    warm[:, :],
                    lhsT=wap,
                    rhs=wrhs,
                    start=(wi == 0),
                    stop=(wi == len(warm_srcs) - 1),
                )

            for b in range(BPC):
                ps = ps_acc.tile([1, MMW], f32, tag="ps")
                nmm = NCH * (CH * FEAT // MMW)
                k = 0
                for c in range(NCH):
                    xt = xts[b * NCH + c]
                    for m in range(CH * FEAT // MMW):
                        nc.tensor.matmul(
                            ps[:, :],
                            lhsT=ones[:, :],
                            rhs=xt[:, m * MMW : (m + 1) * MMW],
                            start=(k == 0),
                            stop=(k == nmm - 1),
                        )
                        k += 1
                # psum row = [even-row partial | odd-row partial]; copy to SBUF
                # (transpose lhsT must be SBUF), then fold even+odd via a pair
                # of accumulating PE transposes per feature-half
                s_row = spool.tile([1, 2 * FEAT], f32, tag="srow")
                nc.scalar.activation(
                    out=s_row[:, :],
                    in_=ps[0:1, :],
                    func=mybir.ActivationFunctionType.Copy,
                    scale=1.0,
                )
                pt = ps_sm.tile([128, BPC], f32, tag="sm")
                for h in range(2):
                    nc.tensor.matmul(
                        pt[:, h : h + 1],
                        lhsT=s_row[0:1, h * 128 : (h + 1) * 128],
                        rhs=one_f[0:1, 0:1],
                        is_transpose=True,
                        start=True,
                        stop=False,
                    )
                    nc.tensor.matmul(
                        pt[:, h : h + 1],
                        lhsT=s_row[0:1, FEAT + h * 128 : FEAT + (h + 1) * 128],
                        rhs=one_f[0:1, 0:1],
                        is_transpose=True,
                        start=False,
                        stop=True,
                    )
                    nc.vector.tensor_copy(
                        out=stq[:, 2 * h + b : 2 * h + b + 1], in_=pt[:, h : h + 1]
                    )

            # out0T = Wv'.T @ ST + bv'   (Wv' = Wv/U, bv' = bv*N/U, host-folded)
            psv = ps_sm.tile([128, BPC], f32, tag="sm")
            nc.tensor.matmul(
                psv[:, :], lhsT=wv_sb[:, 0:MEM], rhs=stq[:, 0:BPC], start=True, stop=False
            )
            nc.tensor.matmul(
                psv[:, :],
                lhsT=wv_sb[:, MEM : 2 * MEM],
                rhs=stq[:, BPC : 2 * BPC],
                start=False,
                stop=True,
            )
            out0 = spool.tile([128, BPC], f32, tag="o0")
            nc.vector.tensor_scalar_add(out=out0[:, :], in0=psv[:, :], scalar1=bv_sb[:, 0:1])

            # outT = relu(Wo.T @ out0T + bo)
            pso = ps_sm.tile([128, BPC], f32, tag="sm")
            nc.tensor.matmul(pso[:, :], lhsT=wo_sb[:, :], rhs=out0[:, :], start=True, stop=True)
            res = spool.tile([128, BPC], f32, tag="res")
            import concourse.mybir as mybir2

            nc.scalar.activation(
                out=res[:, :],
                in_=pso[:, :],
                func=mybir2.ActivationFunctionType.Relu,
                bias=bo_sb[:, 0:1],
                scale=1.0,
            )
            nc.sync.dma_start(out=out_d[:, :], in_=res[:, :])

    if not nc.is_finalized():
        nc.finalize()
    return nc


def kernel(X, mem, Wk, bk, Wv, bv, Wo, bo):
    global _built
    _ensure_axon_hooks()
    from concourse.bass_utils import run_bass_kernel_spmd

    if _built is None:
        _built = _build()
    nc = _built

    X = np.asarray(X, dtype=np.float32)
    Wvs = np.ascontiguousarray(
        (np.asarray(Wv, dtype=np.float32) / float(U)).reshape(2, 128, MEM)
    )
    Wos = np.ascontiguousarray(np.asarray(Wo, dtype=np.float32))
    bvc = np.ascontiguousarray(
        (np.asarray(bv, dtype=np.float32) * (N / float(U))).reshape(MEM, 1)
    )
    boc = np.ascontiguousarray(np.asarray(bo, dtype=np.float32).reshape(MEM, 1))
    onesc = np.ones((128, 1), dtype=np.float32)

    in_maps = [
        {
            "Xs": np.ascontiguousarray(X[i * BPC : (i + 1) * BPC]),
            "Wvs": Wvs,
            "Wos": Wos,
            "bvc": bvc,
            "boc": boc,
            "onesc": onesc,
        }
        for i in range(NCORES)
    ]
    r = run_bass_kernel_spmd(nc, in_maps, list(range(NCORES)))
    kernel._last_results = r

    out = np.empty((B, MEM), dtype=np.float32)
    for i in range(NCORES):
        out[i * BPC : (i + 1) * BPC] = r.results[i]["outT"].T
    return out
